# revision 19
# baseline (speedup 1.0000x reference)
"""3-layer GCN (message passing) + sum-pool + MLP head on 8 Trainium2 cores.

Strategy (all shapes hardcoded; self-contained):
  - Host graph preprocessing: permute nodes into 392 blocks of 128. The
    global table row order is (chunk, core, block): 4 chunk segments of
    15/10/15/9 local blocks per core, so each segment is exactly one
    rank-major AllGather output. Segments 0-1 are gather-half A (25600
    rows), 2-3 half B (int16 idx limit). Per-block edge capacity 1152
    (A-sourced) + 1024 (B-sourced), balanced by a greedy packer.
  - Layer 1 aggregates x*d_inv directly (linearity: A(xW0) = (Ax)W0), so the
    L1 gather table is a replicated input - no dense pre-pass, no AllGather.
  - Tables are bf16 [*, 128]. One 0/1 one-hot routing table (shared by all
    three layers) is HOST-precomputed and streamed in by DMA. GCN
    normalization folds into d_inv^2 epilogues for L1/L2 and a per-column
    d_inv scale before L3's relu (commutes since d_inv > 0). Self-loops are
    a constant identity-matmul chunk.
  - Edge gathers run on 4 SWDGE queues (disjoint GpSimd Q7 core pairs);
    each layer emits its 10 half-A gather calls with a 5-deep lookahead
    ahead of the half-B calls, so the previous boundary's last AllGather
    chunk hides under half-A descriptor generation.
  - The inter-layer AllGather runs as 4 chunk collectives triggered as
    their source blocks finish (3 inside the producing layer, the last at
    the top of the consuming layer); pooled vector is AllReduce'd; tiny
    MLP head runs replicated.
"""
import sys

import numpy as np

for _p in ("/opt/trn_rl_repo", "/root/.axon_site/_ro/trn_rl_repo"):
    if _p not in sys.path:
        sys.path.append(_p)

import ml_dtypes

import concourse.bacc as bacc
import concourse.bass as bass
import concourse.mybir as mybir
import concourse.tile as tile
from concourse.bass_utils import run_bass_kernel_spmd

# ---------------------------------------------------------------- constants
N = 50000                 # real nodes
P = 128
NCORES = 8
BPC = 49                  # blocks per core
NB = BPC * NCORES         # 392 blocks
NP = NB * P               # padded nodes = 50176
ROWS_PC = BPC * P         # 6272 rows per core shard
CH_BLK = [0, 15, 25, 40, 49]          # AllGather chunk bounds (local blocks)
SEG_BASE = [0, 15360, 25600, 40960, 50176]  # global row base per chunk
NCHUNK = 4
HA = SEG_BASE[2]          # 25600 rows in gather-half A (chunks 0-1)
CAPA = 1152               # edge capacity per block, source-half A
CAPB = 1024               # edge capacity per block, source-half B
NCHA = CAPA // P          # 9 chunks from half A
NCHB = CAPB // P          # 8 chunks from half B
CHB = NCHA + NCHB         # 17 edge chunks per block
BATCH_SIZES = [5] * 9 + [4]          # gather batching of the 49 blocks
PRE = 3                   # half-A gather lookahead (batches); < ga bufs
                          # so the lookahead gather never WAR-waits on a
                          # just-issued batch (head-of-line blocks the Pool
                          # sequencer otherwise)
IDX_COLS = ((CAPA + CAPB) // 16) * BPC  # 6664 idx columns (int16, wrap 16)
FW = 128                  # stored table width (bf16)

_CACHED_NC = None
BF16 = ml_dtypes.bfloat16


# ------------------------------------------------------------- host prepro
def _balance_blocks(a_w, b_w, nblocks, cap_a, cap_b):
    """Greedy-pack nodes (with per-node loads a_w/b_w) into blocks of <=128
    nodes with per-half loads <= cap. Returns block id per node position."""
    order = np.argsort(-(a_w + b_w), kind="stable")
    la = np.zeros(nblocks, np.int64)
    lb = np.zeros(nblocks, np.int64)
    cnt = np.zeros(nblocks, np.int64)
    out = np.empty(len(a_w), np.int64)
    for i in order:
        na = la + a_w[i]
        nb_ = lb + b_w[i]
        score = np.maximum(na / cap_a, nb_ / cap_b)
        score[(cnt >= P) | (na > cap_a) | (nb_ > cap_b)] = np.inf
        j = int(np.argmin(score))
        assert np.isfinite(score[j]), "block packing infeasible; raise CAP"
        out[i] = j
        la[j] = na[j]
        lb[j] = nb_[j]
        cnt[j] += 1
    return out


def _cj_to_row(c, j):
    """(core, local block) -> global table row base (numpy-friendly)."""
    k = np.searchsorted(np.array(CH_BLK), j, side="right") - 1
    w = np.array([CH_BLK[i + 1] - CH_BLK[i] for i in range(NCHUNK)])
    base = np.array(SEG_BASE[:NCHUNK])
    return base[k] + (c * w[k] + (j - np.array(CH_BLK)[k])) * P


def _preprocess(x, edge_index):
    src = np.asarray(edge_index[0], np.int64)
    dst = np.asarray(edge_index[1], np.int64)

    deg = np.bincount(dst, minlength=N).astype(np.float64)
    d_inv = 1.0 / np.sqrt(deg + 1.0)

    # ---- split nodes into halves; bias out-edge mass toward half A's
    # larger capacity
    targ_a = CAPA / (CAPA + CAPB)
    out_w = np.bincount(src, minlength=N)
    order = np.argsort(-out_w, kind="stable")
    half = np.zeros(N, np.int8)
    tot = [0.0, 0.0]
    cnti = [0, 0]
    for i in order:
        fa = tot[0] / targ_a
        fb = tot[1] / (1.0 - targ_a)
        h_ = 0 if (fa <= fb and cnti[0] < HA) or cnti[1] >= NP - HA else 1
        half[i] = h_
        tot[h_] += out_w[i]
        cnti[h_] += 1

    # ---- per-node in-loads split by source half
    sh = half[src]
    a_in = np.bincount(dst[sh == 0], minlength=N)
    b_in = np.bincount(dst[sh == 1], minlength=N)

    # ---- pack each half's nodes into its blocks; round-robin blocks over
    # cores (half A -> local blocks 0-24, half B -> 25-48)
    perm_pos = np.empty(N, np.int64)  # node -> global table row
    for h_, nblocks, j0 in ((0, 25 * NCORES, 0), (1, 24 * NCORES, 25)):
        nodes = np.nonzero(half == h_)[0]
        blk = _balance_blocks(a_in[nodes], b_in[nodes], nblocks, CAPA, CAPB)
        o2 = np.argsort(blk, kind="stable")
        sb = blk[o2]
        grp_start = np.searchsorted(sb, np.arange(nblocks), side="left")
        pos_in_grp = np.arange(len(nodes)) - grp_start[sb]
        core = sb % NCORES
        jloc = j0 + sb // NCORES
        perm_pos[nodes[o2]] = _cj_to_row(core, jloc) + pos_in_grp

    # ---- remap edges; (core, local block) of each dst row
    psrc = perm_pos[src]
    pdst = perm_pos[dst]

    seg = np.searchsorted(np.array(SEG_BASE), pdst, side="right") - 1
    w_arr = np.array([CH_BLK[i + 1] - CH_BLK[i] for i in range(NCHUNK)])
    r_in_seg = pdst - np.array(SEG_BASE)[seg]
    dc = r_in_seg // (w_arr[seg] * P)
    dj = np.array(CH_BLK)[seg] + (r_in_seg % (w_arr[seg] * P)) // P
    es = pdst % P               # dst slot
    eh = (psrc >= HA).astype(np.int64)
    eidx = psrc - eh * HA       # gather idx within half

    key = (dc * BPC + dj) * 2 + eh
    order_e = np.argsort(key, kind="stable")
    key_s = key[order_e]
    cnts = np.bincount(key_s, minlength=NB * 2)
    cap_arr = np.where(np.arange(NB * 2) % 2 == 0, CAPA, CAPB)
    assert (cnts <= cap_arr).all(), "block-half overflow; raise CAP"
    starts = np.concatenate([[0], np.cumsum(cnts)[:-1]])
    pos = np.arange(len(key_s)) - starts[key_s]

    # ---- fill per-core device arrays
    idxs = np.zeros((NCORES, 16, IDX_COLS), np.int16)
    oh1 = np.zeros((NCORES, P, BPC * CHB, P), ml_dtypes.float8_e4m3fn)

    g_core = dc[order_e]
    g_j = dj[order_e]
    g_eh = eh[order_e]
    bs_arr = np.array(BATCH_SIZES)
    blk2batch = np.repeat(np.arange(len(bs_arr)), bs_arr)
    batch_blk0 = np.concatenate([[0], np.cumsum(bs_arr)[:-1]])
    g_batch = blk2batch[g_j]
    g_k = g_j - batch_blk0[g_batch]       # block within batch

    # one-hot table: col = j*17 + h*9 + pos//128, row = pos%128, val col es
    col_dw = g_j * CHB + g_eh * NCHA + pos // P
    oh1[g_core, pos % P, col_dw, es[order_e]] = 1

    # idx: batch-grouped wrapped layout; call (t, A) then (t, B)
    batch_col0 = np.concatenate(
        [[0], np.cumsum(((CAPA + CAPB) // 16) * bs_arr)[:-1]])
    cap_eh = np.where(g_eh == 0, CAPA, CAPB)
    call_off = batch_col0[g_batch] + g_eh * (CAPA // 16) * bs_arr[g_batch]
    q = g_k * cap_eh + pos
    idxs[g_core, q % 16, call_off + q // 16] = eidx[order_e]
    idxs_full = np.tile(idxs, (1, 8, 1))  # replicate to 128 partitions

    # ---- bf16 L1 gather table: xg[perm(n), 0:14] = x[n] * d_inv[n]
    xg = np.zeros((NP, FW), BF16)
    xg[perm_pos, :14] = (np.asarray(x, np.float64)
                         * d_inv[:, None]).astype(BF16)
    # per-core shard: its rows from each of the 4 chunk segments
    xg_own = np.concatenate([
        xg[SEG_BASE[k]:SEG_BASE[k + 1]].reshape(
            NCORES, (CH_BLK[k + 1] - CH_BLK[k]) * P, FW)
        for k in range(NCHUNK)], axis=1)

    # ---- per-slot d_inv arrays
    dinv2 = np.zeros((NCORES, P, BPC), np.float32)   # [core][slot, block]
    ac, aj = dc, dj  # reuse mapping helper for all nodes
    seg_n = np.searchsorted(np.array(SEG_BASE), perm_pos, side="right") - 1
    r_n = perm_pos - np.array(SEG_BASE)[seg_n]
    nc_ = r_n // (w_arr[seg_n] * P)
    nj = np.array(CH_BLK)[seg_n] + (r_n % (w_arr[seg_n] * P)) // P
    dinv2[nc_, perm_pos % P, nj] = d_inv * d_inv
    # [core][feat(32), block*128 + slot] broadcast table of d_inv for L3
    dinvb = np.zeros((NCORES, BPC * P), np.float32)
    dinvb[nc_, nj * P + perm_pos % P] = d_inv
    dinvb = np.repeat(dinvb[:, None, :], 32, axis=1)
    return xg, xg_own, idxs_full, oh1, dinv2, dinvb


# ------------------------------------------------------------ device build
def _build_kernel():
    nc = bacc.Bacc("TRN2", target_bir_lowering=False, debug=False,
                   num_swdge_queues=4)
    dt = mybir.dt

    xg = nc.dram_tensor("xg", [NP, FW], dt.bfloat16, kind="ExternalInput")
    xgo = nc.dram_tensor("xgo", [ROWS_PC, FW], dt.bfloat16, kind="ExternalInput")
    w0 = nc.dram_tensor("w0", [14, 128], dt.float32, kind="ExternalInput")
    w1 = nc.dram_tensor("w1", [128, 128], dt.float32, kind="ExternalInput")
    w2p = nc.dram_tensor("w2p", [128, FW], dt.float32, kind="ExternalInput")
    fc11w = nc.dram_tensor("fc11w", [32, 16], dt.float32, kind="ExternalInput")
    fc11b = nc.dram_tensor("fc11b", [16, 1], dt.float32, kind="ExternalInput")
    fc12w = nc.dram_tensor("fc12w", [16, 1], dt.float32, kind="ExternalInput")
    fc12b = nc.dram_tensor("fc12b", [1, 1], dt.float32, kind="ExternalInput")
    ident = nc.dram_tensor("ident", [P, P], dt.bfloat16, kind="ExternalInput")
    dinv2 = nc.dram_tensor("dinv2", [P, BPC], dt.float32, kind="ExternalInput")
    dinvb = nc.dram_tensor("dinvb", [32, BPC * P], dt.float32,
                           kind="ExternalInput")
    idxs = nc.dram_tensor("idxs", [P, IDX_COLS], dt.int16, kind="ExternalInput")
    oh1 = nc.dram_tensor("oh1", [P, BPC * CHB * P], dt.float8e4,
                         kind="ExternalInput")
    out = nc.dram_tensor("out", [1, 1], dt.float32, kind="ExternalOutput")

    bs_arr = np.array(BATCH_SIZES)
    batch_col0 = np.concatenate(
        [[0], np.cumsum(((CAPA + CAPB) // 16) * bs_arr)[:-1]])
    batch_blk0 = np.concatenate([[0], np.cumsum(bs_arr)[:-1]])

    with tile.TileContext(nc) as tc:
        with (
            tc.tile_pool(name="const", bufs=1) as cst,
            tc.tile_pool(name="ga", bufs=6) as gap,
            tc.tile_pool(name="gb", bufs=4) as gbp,
            tc.tile_pool(name="oh", bufs=4) as ohp,
            tc.tile_pool(name="rl", bufs=3) as rlp,
            tc.tile_pool(name="st", bufs=2) as stp,
            tc.tile_pool(name="misc", bufs=1) as msc,
            tc.tile_pool(name="psA", bufs=3, space="PSUM") as psa,
            tc.tile_pool(name="psX", bufs=2, space="PSUM") as psx,
            tc.tile_pool(name="psD", bufs=2, space="PSUM") as psd,
            tc.tile_pool(name="psP", bufs=1, space="PSUM") as psp,
            tc.tile_pool(name="dram", bufs=1, space="DRAM") as drm,
        ):
            # resident constants
            idxs_t = cst.tile([P, IDX_COLS], dt.int16)
            ident_t = cst.tile([P, P], dt.bfloat16)
            dinv2_t = cst.tile([P, BPC], dt.float32)
            dinvb_t = cst.tile([32, BPC * P], dt.float32)
            w0_t = cst.tile([14, 128], dt.float32)
            w1_t = cst.tile([128, 128], dt.float32)
            w2p_t = cst.tile([128, FW], dt.float32)
            fc11w_t = cst.tile([32, 16], dt.float32)
            fc11b_t = cst.tile([16, 1], dt.float32)
            fc12w_t = cst.tile([16, 1], dt.float32)
            fc12b_t = cst.tile([1, 1], dt.float32)
            for t_, d_ in (
                (idxs_t, idxs), (ident_t, ident),
                (dinv2_t, dinv2), (dinvb_t, dinvb),
                (w0_t, w0), (w1_t, w1), (w2p_t, w2p),
                (fc11w_t, fc11w), (fc11b_t, fc11b), (fc12w_t, fc12w),
                (fc12b_t, fc12b),
            ):
                nc.sync.dma_start(t_[:], d_[:])

            # internal DRAM (bf16 tables)
            g2s_t = drm.tile([ROWS_PC, FW], dt.bfloat16)
            g2_t = drm.tile([NP, FW], dt.bfloat16)
            g3s_t = drm.tile([ROWS_PC, FW], dt.bfloat16)
            g3_t = drm.tile([NP, FW], dt.bfloat16)
            pool_in = drm.tile([32, 1], dt.float32)
            pool_out = drm.tile([32, 1], dt.float32, addr_space="Shared")

            pooled_cols = msc.tile([32, BPC], dt.float32)

            def ag_chunk(h_shard, h_full, k):
                nc.gpsimd.collective_compute(
                    "AllGather", mybir.AluOpType.bypass,
                    replica_groups=[list(range(NCORES))],
                    ins=[h_shard[CH_BLK[k] * P : CH_BLK[k + 1] * P, :].opt()],
                    outs=[h_full[SEG_BASE[k] : SEG_BASE[k + 1], :].opt()])

            def emit_ga(h_src, t, bs):
                ic0 = int(batch_col0[t])
                ga = gap.tile([P, NCHA * 5, FW], dt.bfloat16, tag="ga")
                nc.gpsimd.dma_gather(
                    ga[:, : NCHA * bs, :], h_src[0:HA, :],
                    idxs_t[:, ic0 : ic0 + (CAPA // 16) * bs],
                    CAPA * bs, CAPA * bs, FW, single_packet=False,
                    queue_num=t % 2)
                return ga

            def layer(lnum, h_src, h_self, h_shard, h_full, prev_ag3):
                # the deferred boundary chunk goes first: its input (the
                # producer's last dstage) is ready well before this layer's
                # own half-A gathers can dispatch, so it never blocks them
                if prev_ag3 is not None:
                    prev_ag3()
                gas = {}
                for t in range(PRE):
                    gas[t] = emit_ga(h_src, t, BATCH_SIZES[t])
                for t, bs in enumerate(BATCH_SIZES):
                    ic0 = int(batch_col0[t])
                    icb0 = ic0 + (CAPA // 16) * bs
                    b0 = int(batch_blk0[t])
                    ga = gas.pop(t)
                    gb = gbp.tile([P, NCHB * 5, FW], dt.bfloat16, tag="gb")
                    nc.gpsimd.dma_gather(
                        gb[:, : NCHB * bs, :], h_src[HA:NP, :],
                        idxs_t[:, icb0 : icb0 + (CAPB // 16) * bs],
                        CAPB * bs, CAPB * bs, FW, single_packet=False,
                        queue_num=2 + t % 2)
                    if t + PRE < len(BATCH_SIZES):
                        gas[t + PRE] = emit_ga(
                            h_src, t + PRE, BATCH_SIZES[t + PRE])
                    gs = gap.tile([P, 5, FW], dt.bfloat16, tag="gs")
                    nc.sync.dma_start(
                        gs[:, :bs, :],
                        h_self[b0 * P : (b0 + bs) * P, :].rearrange(
                            "(g p) f -> p g f", p=P))
                    ohv = ohp.tile([P, 5 * CHB, P], dt.float8e4, tag="ohv")
                    oh_eng = nc.sync if t % 2 == 0 else nc.scalar
                    oh_eng.dma_start(
                        ohv[:, : bs * CHB, :],
                        oh1[:, b0 * CHB * P : (b0 + bs) * CHB * P].rearrange(
                            "p (c j) -> p c j", j=P))
                    if lnum != 3:
                        dstage = stp.tile([P, 5, FW], dt.bfloat16, tag="dnst")
                    for k in range(bs):
                        j = b0 + k
                        agg = psa.tile([P, P], dt.float32, tag="agg")
                        for c in range(CHB):
                            if c < NCHA:
                                g = ga
                                cc = NCHA * k + c
                            else:
                                g = gb
                                cc = NCHB * k + (c - NCHA)
                            nc.tensor.matmul(
                                agg[:], g[:, cc, :], ohv[:, k * CHB + c, :],
                                start=(c == 0), stop=False)
                        nc.tensor.matmul(
                            agg[:], gs[:, k, :], ident_t[:],
                            start=False, stop=True)

                        if lnum == 1:
                            # ZT = W0^T @ agg_x[0:14]; relu; dense W1; epilogue
                            axs = rlp.tile([14, P], dt.float32, tag="axs")
                            nc.vector.tensor_copy(axs[:], agg[0:14, :])
                            zt = psx.tile([P, P], dt.float32, tag="zt")
                            nc.tensor.matmul(
                                zt[:], w0_t[:], axs[:], start=True, stop=True)
                            rT = rlp.tile([P, P], dt.float32, tag="rT")
                            nc.scalar.activation(
                                rT[:], zt[:],
                                mybir.ActivationFunctionType.Relu)
                            h_ps = psd.tile([P, FW], dt.float32, tag="dnps")
                            nc.tensor.matmul(
                                h_ps[:], rT[:], w1_t[:], start=True, stop=True)
                            nc.vector.tensor_scalar(
                                dstage[:, k, :], h_ps[:],
                                dinv2_t[:, j : j + 1], None,
                                mybir.AluOpType.mult)
                        elif lnum == 2:
                            rT = rlp.tile([P, P], dt.float32, tag="rT")
                            nc.scalar.activation(
                                rT[:], agg[:],
                                mybir.ActivationFunctionType.Relu)
                            h_ps = psd.tile([P, FW], dt.float32, tag="dnps")
                            nc.tensor.matmul(
                                h_ps[:], rT[:], w2p_t[:], start=True, stop=True)
                            nc.vector.tensor_scalar(
                                dstage[:, k, :], h_ps[:],
                                dinv2_t[:, j : j + 1], None,
                                mybir.AluOpType.mult)
                        else:
                            # L3: z3 = d_inv[dst]*agg; relu; sum-pool by col
                            r3w = rlp.tile([32, P], dt.float32, tag="r3w")
                            nc.vector.tensor_tensor(
                                r3w[:], agg[0:32, :],
                                dinvb_t[:, j * P : (j + 1) * P],
                                mybir.AluOpType.mult)
                            r3 = rlp.tile([32, P], dt.float32, tag="r3")
                            nc.scalar.activation(
                                r3[:], r3w[:],
                                mybir.ActivationFunctionType.Relu,
                                accum_out=pooled_cols[:, j : j + 1])
                    if lnum != 3:
                        r0 = b0 * P
                        nc.sync.dma_start(
                            h_shard[r0 : r0 + bs * P, :].rearrange(
                                "(g p) f -> p g f", p=P),
                            dstage[:, :bs, :])
                        if t == 5:
                            ag_chunk(h_shard, h_full, 0)
                        elif t == 7:
                            ag_chunk(h_shard, h_full, 1)
                        elif t == 9:
                            ag_chunk(h_shard, h_full, 2)

            # L1 (x-aggregation)
            layer(1, xg, xgo, g2s_t, g2_t, None)
            # L2 (emits the deferred last AllGather chunk of the L1 boundary)
            layer(2, g2_t, g2s_t, g3s_t, g3_t,
                  lambda: ag_chunk(g2s_t, g2_t, 3))
            # L3 + pooling
            layer(3, g3_t, g3s_t, None, None,
                  lambda: ag_chunk(g3s_t, g3_t, 3))
            pooled = msc.tile([32, 1], dt.float32)
            nc.vector.tensor_reduce(
                pooled[:], pooled_cols[:],
                axis=mybir.AxisListType.X, op=mybir.AluOpType.add)

            # global pool AllReduce + MLP head (replicated)
            nc.sync.dma_start(pool_in[:], pooled[:])
            nc.gpsimd.collective_compute(
                "AllReduce", mybir.AluOpType.add,
                replica_groups=[list(range(NCORES))],
                ins=[pool_in.opt()], outs=[pool_out.opt()])
            pooled_g = msc.tile([32, 1], dt.float32)
            nc.sync.dma_start(pooled_g[:], pool_out[:])
            ps16 = psp.tile([16, 1], dt.float32, tag="mlp")
            nc.tensor.matmul(ps16[:], fc11w_t[:], pooled_g[:], start=True, stop=True)
            a16 = msc.tile([16, 1], dt.float32)
            nc.scalar.activation(
                a16[:], ps16[:], mybir.ActivationFunctionType.Relu,
                bias=fc11b_t[:])
            ps1 = psp.tile([1, 1], dt.float32, tag="mlp")
            nc.tensor.matmul(ps1[:], fc12w_t[:], a16[:], start=True, stop=True)
            o1 = msc.tile([1, 1], dt.float32)
            nc.scalar.activation(
                o1[:], ps1[:], mybir.ActivationFunctionType.Identity,
                bias=fc12b_t[:])
            nc.sync.dma_start(out[:], o1[:])

    nc.compile()
    return nc


def _get_nc():
    global _CACHED_NC
    if _CACHED_NC is None:
        _CACHED_NC = _build_kernel()
    return _CACHED_NC


def _make_in_maps(inputs):
    x = np.asarray(inputs["x"], np.float32)
    edge_index = np.asarray(inputs["edge_index"])
    xg, xg_own, idxs, oh1, dinv2, dinvb = _preprocess(x, edge_index)

    w2p = np.zeros((128, FW), np.float32)
    w2p[:, :32] = np.asarray(inputs["W2"], np.float32)
    common = {
        "xg": xg,
        "w0": np.asarray(inputs["W0"], np.float32),
        "w1": np.asarray(inputs["W1"], np.float32),
        "w2p": w2p,
        "fc11w": np.asarray(inputs["fc11_w"], np.float32),
        "fc11b": np.asarray(inputs["fc11_b"], np.float32).reshape(16, 1),
        "fc12w": np.asarray(inputs["fc12_w"], np.float32),
        "fc12b": np.asarray(inputs["fc12_b"], np.float32).reshape(1, 1),
        "ident": np.eye(P, dtype=BF16),
    }
    return [
        {**common, "xgo": np.ascontiguousarray(xg_own[c]), "idxs": idxs[c],
         "oh1": oh1[c].reshape(P, BPC * CHB * P),
         "dinv2": dinv2[c], "dinvb": dinvb[c]}
        for c in range(NCORES)
    ]


def run(trace=False, _inputs=None, **inputs):
    if _inputs is not None:
        inputs = _inputs
    in_maps = _make_in_maps(inputs)
    nc = _get_nc()
    res = run_bass_kernel_spmd(
        nc, in_maps, core_ids=list(range(NCORES)), trace=trace)
    y = np.asarray(res.results[0]["out"], np.float32).reshape(1)
    return y, res


def kernel(**inputs) -> np.ndarray:
    y, _ = run(**inputs)
    return y
